# revision 10
# baseline (speedup 1.0000x reference)
"""DualEncoder (two shared-weight LSTM encoders + bilinear score) on 8 trn2
NeuronCores.

Sharding: 8-way tensor parallelism over the 4H gate dimension. Every core
holds the full batch (64 ctx + 64 resp sequences = 128 rows) and a 512-wide
gate slice in order [i|f|g|o] (128 each). Per step: gates = bias + x_t @
Wih_k^T + h_{t-1} @ Whh_k^T accumulated in a 4-deep PSUM ring; the input
projection for step t+2 runs ahead of time as PE filler so the tensor engine
never idles (keeps the HAM clock-gate warm). Recurrent matmuls are split into
an [i|f|g] group (N=384) and an [o] group (N=128) so activations start
early. Embedding rows are gathered with an f32->bf16 cast in the DMA; X
transposes are bf16 PE transposes. The new h slice is PE-transposed, DVE-cast
to bf16 and remote-broadcast to all cores. sigmoid(diag(C @ M @ R^T)) is
computed replicated at the end.
"""

import os

import numpy as np

N_CORES = 8
B = 64
T = 160
E = 512
H = 1024
V = 32000
GS = 512          # gate-slice width per core
S = 2 * B         # 128 sequences (ctx rows 0:64, resp rows 64:128)
ARR_PER_STEP = 14  # 7 remote senders x 2 sem incs each
XR = 4            # gather prefetch depth (xraw slots)
XS = 3            # transposed-X buffer depth (xt_sb slots)
NA = 384          # i|f|g column group width
NB = 128          # o column group width

LAST_EXEC_NS = None
_NC_CACHE = {}

# gate-chunk order inside the core's 512-wide slice: [i | f | g | o]
GATE_OFF = [0, H, 2 * H, 3 * H]


def _build(t_steps=T):
    from contextlib import ExitStack

    import concourse.bacc as bacc
    import concourse.bass as bass
    import concourse.mybir as mybir
    from concourse import masks

    f32 = mybir.dt.float32
    bf16 = mybir.dt.bfloat16
    i32 = mybir.dt.int32

    nc = bacc.Bacc("TRN2", debug=False, num_devices=N_CORES)

    d_ctx = nc.dram_tensor("contexts", [B, T], i32, kind="ExternalInput")
    d_rsp = nc.dram_tensor("responses", [B, T], i32, kind="ExternalInput")
    d_emb = nc.dram_tensor("emb", [V, E], f32, kind="ExternalInput")
    d_wih = nc.dram_tensor("Wih", [4 * H, E], f32, kind="ExternalInput")
    d_whh = nc.dram_tensor("Whh", [4 * H, H], f32, kind="ExternalInput")
    d_bih = nc.dram_tensor("bih", [1, 4 * H], f32, kind="ExternalInput")
    d_bhh = nc.dram_tensor("bhh", [1, 4 * H], f32, kind="ExternalInput")
    d_m = nc.dram_tensor("M", [H, H], f32, kind="ExternalInput")
    d_out = nc.dram_tensor("out", [1, B], f32, kind="ExternalOutput")

    arr = nc.monotonic_semaphore(0)

    es = ExitStack()
    sb = lambda name, shape, dt: es.enter_context(nc.sbuf_tensor(name, shape, dt))
    psa = lambda name, shape: es.enter_context(nc.psum_tensor(name, shape, f32))
    sem = lambda name: es.enter_context(nc.semaphore(name))

    tok = sb("tok", [S, T], i32)
    whhT = sb("whhT", [128, 8 * GS], bf16)
    wihT = sb("wihT", [128, 4 * GS], bf16)
    msb = sb("msb", [128, 8 * H], bf16)
    stagW = sb("stagW", [128, 4 * H], f32)
    stagI = sb("stagI", [128, 4 * E], f32)
    ident = sb("ident", [128, 128], f32)
    ident16 = sb("ident16", [128, 128], bf16)
    ones1 = sb("ones1", [1, 128], bf16)
    ones128 = sb("ones128", [128, 1], f32)
    bias = sb("bias", [1, GS], f32)
    btmp = sb("btmp", [1, GS], f32)
    bias16 = sb("bias16", [1, GS], bf16)
    xraw = sb("xraw", [128, XR * E], f32)
    xt_sb = sb("xt_sb", [128, XS * E], bf16)
    hbuf = sb("hbuf", [128, 2 * H], bf16)
    h_sb = sb("h_sb", [128, 2 * 128], f32)
    c_sb = sb("c_sb", [128, 128], f32)
    sig_sb = sb("sig_sb", [128, 2 * 384], f32)  # per parity: [i|f|o] 128 each
    tg_sb = sb("tg_sb", [128, 2 * 128], f32)
    tc_sb = sb("tc_sb", [128, 2 * 128], f32)
    t1_sb = sb("t1_sb", [128, 128], f32)
    t2_sb = sb("t2_sb", [128, 128], f32)
    rh_sb = sb("rh_sb", [128, 8 * B], f32)
    zw_sb = sb("zw_sb", [128, 8 * B], f32)
    out_sb = sb("out_sb", [1, B], f32)

    gates_ps = psa("gates_ps", [128, 4 * GS])   # 4-bank ring, bank r = t%4
    xt_ps = psa("xt_ps", [128, 2 * E])          # f32 transpose scratch (2 banks)
    misc_ps = psa("misc_ps", [128, 512])        # ht parities @0/128, s @[0:1,256:320]
    z_ps = psa("z_ps", [128, 8 * B])

    s_sync = sem("s_sync")
    s_gdma = sem("s_gdma")
    s_gset = sem("s_gset")
    s_gather = sem("s_gather")
    s_wtp = sem("s_wtp")
    s_wtc = sem("s_wtc")
    s_bias = sem("s_bias")
    s_xtp = sem("s_xtp")      # X(u) transposed -> u+1
    s_xtc = sem("s_xtc")      # X(u) cast to xt_sb -> u+1
    s_gA = sem("s_gA")        # gates i|f|g of step t done -> t+1
    s_gB = sem("s_gB")        # gates o of step t done -> t+1
    s_if = sem("s_if")
    s_tg = sem("s_tg")
    s_so = sem("s_so")
    s_c = sem("s_c")
    s_tc = sem("s_tc")
    s_h = sem("s_h")
    s_htp = sem("s_htp")
    s_htc = sem("s_htc")
    s_send = sem("s_send")
    s_prep = sem("s_prep")
    s_z = sem("s_z")
    s_zmul = sem("s_zmul")
    s_zred = sem("s_zred")
    s_out = sem("s_out")
    s_fin = sem("s_fin")

    p_last = (t_steps - 1) % 2
    look_g = min(XR, t_steps)
    look_x = min(3, t_steps)

    with nc.Block() as block:

        # ---------------- SYNC (HWDGE): setup loads + final store ----------
        @block.sync
        def _(sync):
            pid = sync.partition_id()
            sync.dma_start(tok[0:B, :], d_ctx[:, :]).then_inc(s_sync, 16)
            sync.dma_start(tok[B:S, :], d_rsp[:, :]).then_inc(s_sync, 16)
            for m in range(4):  # -> 96
                sync.dma_start(
                    stagW[:, H * m : H * (m + 1)],
                    d_whh[bass.ds(pid * 128 + GATE_OFF[m], 128), :],
                ).then_inc(s_sync, 16)
            for m in range(4):  # -> 160
                sync.dma_start(
                    stagI[:, E * m : E * (m + 1)],
                    d_wih[bass.ds(pid * 128 + GATE_OFF[m], 128), :],
                ).then_inc(s_sync, 16)
            for m in range(4):  # -> 224
                sync.dma_start(
                    bias[:, 128 * m : 128 * (m + 1)],
                    d_bih[:, bass.ds(pid * 128 + GATE_OFF[m], 128)],
                ).then_inc(s_sync, 16)
            for m in range(4):  # -> 288
                sync.dma_start(
                    btmp[:, 128 * m : 128 * (m + 1)],
                    d_bhh[:, bass.ds(pid * 128 + GATE_OFF[m], 128)],
                ).then_inc(s_sync, 16)
            sync.wait_ge(s_out, 1)
            sync.dma_start(d_out[:, :], out_sb[:, :]).then_inc(s_fin, 16)
            sync.wait_ge(s_fin, 16)

        # ---------------- GPSIMD: setup, gathers, broadcasts ---------------
        @block.gpsimd
        def _(gpsimd):
            pid = gpsimd.partition_id()
            masks.make_identity(nc, ident[:, :])
            gpsimd.memset(ones1[:, :], 1.0)
            gpsimd.memset(ones128[:, :], 1.0)
            gpsimd.memset(c_sb[:, :], 0.0)
            gpsimd.sem_inc(s_gset, 1)
            for i in range(8):  # M load, f32 -> bf16 cast via SWDGE (-> 128)
                gpsimd.dma_start(
                    msb[:, H * i : H * (i + 1)], d_m[128 * i : 128 * (i + 1), :]
                ).then_inc(s_gdma, 16)

            gpsimd.wait_ge(s_sync, 32)
            for u in range(look_g):  # prologue gathers (bf16 cast in DMA)
                gpsimd.indirect_dma_start(
                    out=xraw[:, E * (u % XR) : E * (u % XR + 1)],
                    out_offset=None,
                    in_=d_emb[:, :],
                    in_offset=bass.IndirectOffsetOnAxis(
                        ap=tok[:, u : u + 1], axis=0
                    ),
                ).then_inc(s_gather, 16)

            rdests = [None] + [(0, k) for k in range(1, N_CORES)]
            for t in range(t_steps):
                p = t % 2
                if t + XR < t_steps:
                    # xraw slot (t+XR)%XR free once X(t) transposed
                    gpsimd.wait_ge(s_xtp, t + 1)
                    if t >= 1:
                        # keep the gather off the SDMA engines while our own
                        # previous broadcast drains
                        gpsimd.wait_ge(s_send, 16 * t)
                    gpsimd.indirect_dma_start(
                        out=xraw[:, E * ((t + XR) % XR) : E * ((t + XR) % XR + 1)],
                        out_offset=None,
                        in_=d_emb[:, :],
                        in_offset=bass.IndirectOffsetOnAxis(
                            ap=tok[:, t + XR : t + XR + 1], axis=0
                        ),
                    ).then_inc(s_gather, 16)
                own = hbuf[:, bass.ds(H * p + pid * 128, 128)]
                gpsimd.remote_dma_broadcast(
                    out_ap=own,
                    in_ap=own,
                    remote_sem=arr.sem(),
                    local_sem=s_send,
                    rdests=rdests,
                ).then_inc(s_prep, 1)
                gpsimd.wait_ge(s_prep, t + 1)
                gpsimd.wait_ge(s_htc, t + 1)
                gpsimd.trigger_dma(count=1)

        # ---------------- PE: transposes + matmuls --------------------------
        @block.tensor
        def _(pe):
            pe.wait_ge(s_sync, 160)
            pe.wait_ge(s_gset, 1)
            # weight transposes: Whh 4 groups of 8, Wih 2 groups of 8 (xt_ps
            # as 8-tile scratch)
            for g in range(4):
                if g >= 1:
                    pe.wait_ge(s_wtc, g)
                for j in range(8):
                    ins = nc.tensor.transpose(
                        xt_ps[:, 128 * j : 128 * (j + 1)],
                        stagW[:, H * g + 128 * j : H * g + 128 * (j + 1)],
                        ident[:, :],
                    )
                    if j == 7:
                        ins.then_inc(s_wtp, 1)
            for g2 in range(2):
                pe.wait_ge(s_wtc, 4 + g2)
                for r in range(8):
                    idx = 8 * g2 + r
                    m, e = idx // 4, idx % 4
                    ins = nc.tensor.transpose(
                        xt_ps[:, 128 * r : 128 * (r + 1)],
                        stagI[:, E * m + 128 * e : E * m + 128 * (e + 1)],
                        ident[:, :],
                    )
                    if r == 7:
                        ins.then_inc(s_wtp, 1)

            # X transpose prologue: X(0..look_x-1), bf16
            for u in range(look_x):
                pe.wait_ge(s_gather, 16 * (u + 1))
                if u < 2:
                    pe.wait_ge(s_wtc, 6)  # xt_ps scratch free of weight use
                else:
                    pe.wait_ge(s_xtc, u - 1)  # parity u%2 free (X(u-2) copied)
                for e in range(4):
                    ins = nc.tensor.transpose(
                        xt_ps[:, E * (u % 2) + 128 * e : E * (u % 2) + 128 * (e + 1)],
                        xraw[:, E * (u % XR) + 128 * e : E * (u % XR) + 128 * (e + 1)],
                        ident[:, :],
                    )
                    if e == 3:
                        ins.then_inc(s_xtp, 1)

            pe.wait_ge(s_bias, 1)

            def proj(step, full_stop):
                """bias + X(step) @ WihT into ring bank step%4."""
                r = step % 4
                sl = step % XS
                nc.tensor.matmul(
                    gates_ps[:, GS * r : GS * (r + 1)], ones1[:, :], bias16[:, :],
                    start=True, stop=False, skip_group_check=True,
                )
                for e in range(4):
                    mm = nc.tensor.matmul(
                        gates_ps[:, GS * r : GS * (r + 1)],
                        xt_sb[:, E * sl + 128 * e : E * sl + 128 * (e + 1)],
                        wihT[:, GS * e : GS * (e + 1)],
                        start=False,
                        stop=full_stop and (e == 3),
                        skip_group_check=True,
                    )
                    if full_stop and e == 3:
                        mm.then_inc(s_gA, 1)
                if full_stop:
                    pe.sem_inc(s_gB, 1)

            # prologue projections: step 0 (complete gates: no recurrent term),
            # step 1 (left open for its recurrent matmuls)
            pe.wait_ge(s_xtc, 1)
            proj(0, full_stop=True)
            if t_steps > 1:
                pe.wait_ge(s_xtc, 2)
                proj(1, full_stop=False)

            for t in range(t_steps):
                r = t % 4
                # recurrent matmuls for step t
                if t >= 1:
                    pm = (t - 1) % 2
                    pe.wait_ge(s_htc, t)
                    pe.wait_ge(arr.sem(), ARR_PER_STEP * t)
                    for j in range(8):
                        mm = nc.tensor.matmul(
                            gates_ps[:, GS * r : GS * (r + 1)],
                            hbuf[:, H * pm + 128 * j : H * pm + 128 * (j + 1)],
                            whhT[:, GS * j : GS * (j + 1)],
                            start=False,
                            stop=(j == 7),
                            skip_group_check=True,
                        )
                        if j == 7:
                            mm.then_inc(s_gA, 1)
                    pe.sem_inc(s_gB, 1)
                # transpose X(t+3): its gather was issued an iteration ago
                if t + 3 < t_steps:
                    pe.wait_ge(s_gather, 16 * (t + 4))
                    pe.wait_ge(s_xtc, t + 2)  # parity free (X(t+1) copied)
                    for e in range(4):
                        u = t + 3
                        ins = nc.tensor.transpose(
                            xt_ps[:, E * (u % 2) + 128 * e : E * (u % 2) + 128 * (e + 1)],
                            xraw[:, E * (u % XR) + 128 * e : E * (u % XR) + 128 * (e + 1)],
                            ident[:, :],
                        )
                        if e == 3:
                            ins.then_inc(s_xtp, 1)
                # transpose the new h slice as soon as it is ready
                pe.wait_ge(s_h, t + 1)
                p = t % 2
                nc.tensor.transpose(
                    misc_ps[:, 128 * p : 128 * (p + 1)],
                    h_sb[:, 128 * p : 128 * (p + 1)],
                    ident[:, :],
                ).then_inc(s_htp, 1)
                # 1-wide-lhsT separator MM: keeps a non-FWL matmul between the
                # fp32 transpose above and the next 128-wide bf16 weight load
                # (FWL-after-fp32HI is a known HW-hang hazard)
                nc.tensor.matmul(
                    z_ps[:, 0:B], ones1[:, :], bias16[:, 0:B],
                    start=True, stop=True, skip_group_check=True,
                )
                # input projection for step t+2 (filler for the broadcast window)
                if t + 2 < t_steps:
                    pe.wait_ge(s_xtc, t + 3)
                    proj(t + 2, full_stop=False)

            # ---------------- bilinear epilogue ----------------
            pe.wait_ge(arr.sem(), ARR_PER_STEP * t_steps)
            pe.wait_ge(s_htc, t_steps)
            pe.wait_ge(s_gdma, 128)
            for jm in range(8):
                for im in range(8):
                    mm = nc.tensor.matmul(
                        z_ps[:, B * jm : B * (jm + 1)],
                        msb[:, H * im + 128 * jm : H * im + 128 * (jm + 1)],
                        hbuf[:, H * p_last + 128 * im : H * p_last + 128 * im + B],
                        start=(im == 0),
                        stop=(im == 7),
                    )
                    if jm == 7 and im == 7:
                        mm.then_inc(s_z, 1)
            pe.wait_ge(s_zmul, 1)
            for jm in range(8):
                mm = nc.tensor.matmul(
                    misc_ps[0:1, 256:320],
                    ones128[:, :],
                    zw_sb[:, B * jm : B * (jm + 1)],
                    start=(jm == 0),
                    stop=(jm == 7),
                )
                if jm == 7:
                    mm.then_inc(s_zred, 1)

        # ---------------- ACT (scalar): activations ----------------
        @block.scalar
        def _(act):
            import concourse.mybir as mybir

            AF = mybir.ActivationFunctionType
            for t in range(t_steps):
                r = t % 4
                p = t % 2
                act.wait_ge(s_gA, t + 1)
                nc.scalar.activation(
                    sig_sb[:, 384 * p : 384 * p + 256],  # i|f
                    gates_ps[:, GS * r : GS * r + 256],
                    AF.Sigmoid,
                ).then_inc(s_if, 1)
                nc.scalar.activation(
                    tg_sb[:, 128 * p : 128 * (p + 1)],  # g
                    gates_ps[:, GS * r + 256 : GS * r + 384],
                    AF.Tanh,
                ).then_inc(s_tg, 1)
                act.wait_ge(s_gB, t + 1)
                ins = nc.scalar.activation(
                    sig_sb[:, 384 * p + 256 : 384 * p + 384],  # o
                    gates_ps[:, GS * r + 384 : GS * (r + 1)],
                    AF.Sigmoid,
                )
                ins.then_inc(s_so, 1)
                act.wait_ge(s_c, t + 1)
                nc.scalar.activation(
                    tc_sb[:, 128 * p : 128 * (p + 1)],
                    c_sb[:, :],
                    AF.Tanh,
                ).then_inc(s_tc, 1)

            # epilogue sigmoid
            act.wait_ge(s_zred, 1)
            nc.scalar.activation(
                out_sb[:, :], misc_ps[0:1, 256:320], AF.Sigmoid
            ).then_inc(s_out, 1)

        # ---------------- DVE (vector): copies + elementwise ----------------
        @block.vector
        def _(dve):
            dve_pid = dve.partition_id()

            # weight transpose copies (cast f32 -> bf16)
            for g in range(4):
                dve.wait_ge(s_wtp, g + 1)
                for j in range(8):
                    ins = nc.vector.tensor_copy(
                        whhT[:, GS * j + 128 * g : GS * j + 128 * (g + 1)],
                        xt_ps[:, 128 * j : 128 * (j + 1)],
                    )
                    if j == 7:
                        ins.then_inc(s_wtc, 1)
            for g2 in range(2):
                dve.wait_ge(s_wtp, 5 + g2)
                for r in range(8):
                    idx = 8 * g2 + r
                    m, e = idx // 4, idx % 4
                    ins = nc.vector.tensor_copy(
                        wihT[:, GS * e + 128 * m : GS * e + 128 * (m + 1)],
                        xt_ps[:, 128 * r : 128 * (r + 1)],
                    )
                    if r == 7:
                        ins.then_inc(s_wtc, 1)
            # bias add
            dve.wait_ge(s_sync, 288)
            nc.vector.tensor_add(bias[:, :], bias[:, :], btmp[:, :])
            nc.vector.tensor_copy(bias16[:, :], bias[:, :]).then_inc(s_bias, 1)
            # prologue X casts (one 512-wide cast per step)
            for u in range(look_x):
                dve.wait_ge(s_xtp, u + 1)
                nc.vector.tensor_copy(
                    xt_sb[:, E * (u % XS) : E * (u % XS + 1)],
                    xt_ps[:, E * (u % 2) : E * (u % 2 + 1)],
                ).then_inc(s_xtc, 1)
            dve.wait_ge(s_gset, 1)

            for t in range(t_steps):
                p = t % 2
                dve.wait_ge(s_if, t + 1)
                nc.vector.tensor_mul(
                    t1_sb[:, :],
                    sig_sb[:, 384 * p + 128 : 384 * p + 256],  # f
                    c_sb[:, :],
                )
                dve.wait_ge(s_tg, t + 1)
                nc.vector.tensor_mul(
                    t2_sb[:, :],
                    sig_sb[:, 384 * p : 384 * p + 128],  # i
                    tg_sb[:, 128 * p : 128 * (p + 1)],
                )
                nc.vector.tensor_add(c_sb[:, :], t1_sb[:, :], t2_sb[:, :]).then_inc(
                    s_c, 1
                )
                # cast X(t+3) during the tanh(c) window
                if t + 3 < t_steps:
                    u = t + 3
                    dve.wait_ge(s_xtp, u + 1)
                    nc.vector.tensor_copy(
                        xt_sb[:, E * (u % XS) : E * (u % XS + 1)],
                        xt_ps[:, E * (u % 2) : E * (u % 2 + 1)],
                    ).then_inc(s_xtc, 1)
                dve.wait_ge(s_tc, t + 1)
                dve.wait_ge(s_so, t + 1)
                nc.vector.tensor_mul(
                    h_sb[:, 128 * p : 128 * (p + 1)],
                    sig_sb[:, 384 * p + 256 : 384 * p + 384],  # o
                    tc_sb[:, 128 * p : 128 * (p + 1)],
                ).then_inc(s_h, 1)
                # cast h^T into hbuf (bf16) once PE transposed it; make sure
                # the send of step t-2 (same parity) has drained first
                dve.wait_ge(s_htp, t + 1)
                if t >= 2:
                    dve.wait_ge(s_send, 16 * (t - 1))
                nc.vector.tensor_copy(
                    hbuf[:, bass.ds(H * p + dve_pid * 128, 128)],
                    misc_ps[:, 128 * p : 128 * (p + 1)],
                ).then_inc(s_htc, 1)

            # epilogue: rh cast + elementwise mul
            dve.wait_ge(s_z, 1)
            for jm in range(8):
                nc.vector.tensor_copy(
                    rh_sb[:, B * jm : B * (jm + 1)],
                    hbuf[:, H * p_last + 128 * jm + B : H * p_last + 128 * (jm + 1)],
                )
            for jm in range(8):
                ins = nc.vector.tensor_mul(
                    zw_sb[:, B * jm : B * (jm + 1)],
                    z_ps[:, B * jm : B * (jm + 1)],
                    rh_sb[:, B * jm : B * (jm + 1)],
                )
                if jm == 7:
                    ins.then_inc(s_zmul, 1)

    es.close()
    nc.compile()
    return nc


def _get_nc(t_steps=T):
    if t_steps not in _NC_CACHE:
        _NC_CACHE[t_steps] = _build(t_steps)
    return _NC_CACHE[t_steps]


def kernel(**inputs):
    global LAST_EXEC_NS
    from concourse.bass_utils import run_bass_kernel_spmd

    t_steps = int(os.environ.get("BASS_KERNEL_TSTEPS", str(T)))
    nc = _get_nc(t_steps)
    in_map = {
        "contexts": np.ascontiguousarray(np.asarray(inputs["contexts"], np.int32)),
        "responses": np.ascontiguousarray(np.asarray(inputs["responses"], np.int32)),
        "emb": np.ascontiguousarray(np.asarray(inputs["emb"], np.float32)),
        "Wih": np.ascontiguousarray(np.asarray(inputs["Wih"], np.float32)),
        "Whh": np.ascontiguousarray(np.asarray(inputs["Whh"], np.float32)),
        "bih": np.ascontiguousarray(
            np.asarray(inputs["bih"], np.float32).reshape(1, 4 * H)
        ),
        "bhh": np.ascontiguousarray(
            np.asarray(inputs["bhh"], np.float32).reshape(1, 4 * H)
        ),
        "M": np.ascontiguousarray(np.asarray(inputs["M"], np.float32)),
    }
    res = run_bass_kernel_spmd(
        nc,
        [dict(in_map) for _ in range(N_CORES)],
        core_ids=list(range(N_CORES)),
        trace=bool(int(os.environ.get("BASS_KERNEL_TRACE", "0"))),
        trace_cores=(
            list(range(N_CORES))
            if int(os.environ.get("BASS_KERNEL_TRACE_ALL", "0"))
            else None
        ),
    )
    LAST_EXEC_NS = res.exec_time_ns
    return res.results[0]["out"].reshape(B).astype(np.float32)


# revision 14
# speedup vs baseline: 1.0251x; 1.0251x over previous
"""DualEncoder (two shared-weight LSTM encoders + bilinear score) on 8 trn2
NeuronCores.

Sharding: 8-way tensor parallelism over the 4H gate dimension. Every core
holds the full batch (64 ctx + 64 resp sequences = 128 rows) and a 512-wide
gate slice in order [i|f|g|o] (128 each). Per step: gates = bias + x_t @
Wih_k^T + h_{t-1} @ Whh_k^T accumulated in a 4-deep PSUM ring; the input
projection for step t+2 runs ahead of time as PE filler, and staged
warm-keeping matmuls bridge the arrival wait so the PE idle gap stays under
the HAM clock-gate re-throttle window. Activations are widened (one sigmoid
over [i|f], one tanh for g, one sigmoid for o); the new h slice is
PE-transposed, DVE-cast to bf16, and remote-broadcast to every core.
sigmoid(diag(C @ M @ R^T)) is computed replicated at the end.
"""

import os

import numpy as np

N_CORES = 8
B = 64
T = 160
E = 512
H = 1024
V = 32000
GS = 512          # gate-slice width per core
S = 2 * B         # 128 sequences (ctx rows 0:64, resp rows 64:128)
ARR_PER_STEP = 14  # 7 remote senders x 2 sem incs each
XR = 4            # gather prefetch depth (xraw slots)
XS = 3            # transposed-X buffer depth (xt_sb slots)
NA = 384          # i|f|g column group width
NB = 128          # o column group width

LAST_EXEC_NS = None
_NC_CACHE = {}

# gate-chunk order inside the core's 512-wide slice: [i | f | g | o]
GATE_OFF = [0, H, 2 * H, 3 * H]


def _build(t_steps=T):
    from contextlib import ExitStack

    import concourse.bacc as bacc
    import concourse.bass as bass
    import concourse.mybir as mybir
    from concourse import masks

    f32 = mybir.dt.float32
    bf16 = mybir.dt.bfloat16
    i32 = mybir.dt.int32

    nc = bacc.Bacc("TRN2", debug=False, num_devices=N_CORES)

    d_ctx = nc.dram_tensor("contexts", [B, T], i32, kind="ExternalInput")
    d_rsp = nc.dram_tensor("responses", [B, T], i32, kind="ExternalInput")
    d_emb = nc.dram_tensor("emb", [V, E], f32, kind="ExternalInput")
    d_wih = nc.dram_tensor("Wih", [4 * H, E], f32, kind="ExternalInput")
    d_whh = nc.dram_tensor("Whh", [4 * H, H], f32, kind="ExternalInput")
    d_bih = nc.dram_tensor("bih", [1, 4 * H], f32, kind="ExternalInput")
    d_bhh = nc.dram_tensor("bhh", [1, 4 * H], f32, kind="ExternalInput")
    d_m = nc.dram_tensor("M", [H, H], f32, kind="ExternalInput")
    d_out = nc.dram_tensor("out", [1, B], f32, kind="ExternalOutput")

    arr = nc.monotonic_semaphore(0)

    es = ExitStack()
    sb = lambda name, shape, dt: es.enter_context(nc.sbuf_tensor(name, shape, dt))
    psa = lambda name, shape: es.enter_context(nc.psum_tensor(name, shape, f32))
    sem = lambda name: es.enter_context(nc.semaphore(name))

    tok = sb("tok", [S, T], i32)
    whhT = sb("whhT", [128, 8 * GS], bf16)
    wihT = sb("wihT", [128, 4 * GS], bf16)
    msb = sb("msb", [128, 8 * H], bf16)
    stagW = sb("stagW", [128, 4 * H], f32)
    stagI = sb("stagI", [128, 4 * E], f32)
    ident = sb("ident", [128, 128], f32)
    ident16 = sb("ident16", [128, 128], bf16)
    ones1 = sb("ones1", [1, 128], bf16)
    ones128 = sb("ones128", [128, 1], f32)
    bias = sb("bias", [1, GS], f32)
    btmp = sb("btmp", [1, GS], f32)
    bias16 = sb("bias16", [1, GS], bf16)
    xraw = sb("xraw", [128, XR * E], f32)
    xt_sb = sb("xt_sb", [128, XS * E], bf16)
    hbuf = sb("hbuf", [128, 2 * H], bf16)
    h_sb = sb("h_sb", [128, 2 * 128], f32)
    c_sb = sb("c_sb", [128, 128], f32)
    sig_sb = sb("sig_sb", [128, 2 * 384], f32)  # per parity: [i|f|o] 128 each
    tg_sb = sb("tg_sb", [128, 2 * 128], f32)
    tc_sb = sb("tc_sb", [128, 2 * 128], f32)
    t1_sb = sb("t1_sb", [128, 128], f32)
    t2_sb = sb("t2_sb", [128, 128], f32)
    rh_sb = sb("rh_sb", [128, 8 * B], f32)
    zw_sb = sb("zw_sb", [128, 8 * B], f32)
    out_sb = sb("out_sb", [1, B], f32)

    gates_ps = psa("gates_ps", [128, 4 * GS])   # 4-bank ring, bank r = t%4
    xt_ps = psa("xt_ps", [128, 2 * E])          # f32 transpose scratch (2 banks)
    misc_ps = psa("misc_ps", [128, 512])        # ht parities @0/128, s @[0:1,256:320]
    z_ps = psa("z_ps", [128, 8 * B])

    s_sync = sem("s_sync")
    s_gdma = sem("s_gdma")
    s_gset = sem("s_gset")
    s_gather = sem("s_gather")
    s_wtp = sem("s_wtp")
    s_wtc = sem("s_wtc")
    s_bias = sem("s_bias")
    s_xtp = sem("s_xtp")      # X(u) transposed -> u+1
    s_xtc = sem("s_xtc")      # X(u) cast to xt_sb -> u+1
    s_gA = sem("s_gA")        # gates i|f|g of step t done -> t+1
    s_gB = sem("s_gB")        # gates o of step t done -> t+1
    s_if = sem("s_if")
    s_tg = sem("s_tg")
    s_so = sem("s_so")
    s_c = sem("s_c")
    s_tc = sem("s_tc")
    s_h = sem("s_h")
    s_htp = sem("s_htp")
    s_htc = sem("s_htc")
    s_send = sem("s_send")
    s_prep = sem("s_prep")
    s_z = sem("s_z")
    s_zmul = sem("s_zmul")
    s_zred = sem("s_zred")
    s_out = sem("s_out")
    s_fin = sem("s_fin")

    p_last = (t_steps - 1) % 2
    look_g = min(XR, t_steps)
    look_x = min(3, t_steps)

    with nc.Block() as block:

        # ---------------- SYNC (HWDGE): setup loads + final store ----------
        @block.sync
        def _(sync):
            pid = sync.partition_id()
            sync.dma_start(tok[0:B, :], d_ctx[:, :]).then_inc(s_sync, 16)
            sync.dma_start(tok[B:S, :], d_rsp[:, :]).then_inc(s_sync, 16)
            for m in range(4):  # -> 96
                sync.dma_start(
                    stagW[:, H * m : H * (m + 1)],
                    d_whh[bass.ds(pid * 128 + GATE_OFF[m], 128), :],
                ).then_inc(s_sync, 16)
            for m in range(4):  # -> 160
                sync.dma_start(
                    stagI[:, E * m : E * (m + 1)],
                    d_wih[bass.ds(pid * 128 + GATE_OFF[m], 128), :],
                ).then_inc(s_sync, 16)
            for m in range(4):  # -> 224
                sync.dma_start(
                    bias[:, 128 * m : 128 * (m + 1)],
                    d_bih[:, bass.ds(pid * 128 + GATE_OFF[m], 128)],
                ).then_inc(s_sync, 16)
            for m in range(4):  # -> 288
                sync.dma_start(
                    btmp[:, 128 * m : 128 * (m + 1)],
                    d_bhh[:, bass.ds(pid * 128 + GATE_OFF[m], 128)],
                ).then_inc(s_sync, 16)
            sync.wait_ge(s_out, 1)
            sync.dma_start(d_out[:, :], out_sb[:, :]).then_inc(s_fin, 16)
            sync.wait_ge(s_fin, 16)

        # ---------------- GPSIMD: setup, gathers, broadcasts ---------------
        @block.gpsimd
        def _(gpsimd):
            pid = gpsimd.partition_id()
            masks.make_identity(nc, ident[:, :])
            gpsimd.memset(ones1[:, :], 1.0)
            gpsimd.memset(ones128[:, :], 1.0)
            gpsimd.memset(c_sb[:, :], 0.0)
            gpsimd.sem_inc(s_gset, 1)
            for i in range(8):  # M load, f32 -> bf16 cast via SWDGE (-> 128)
                gpsimd.dma_start(
                    msb[:, H * i : H * (i + 1)], d_m[128 * i : 128 * (i + 1), :]
                ).then_inc(s_gdma, 16)

            gpsimd.wait_ge(s_sync, 32)
            for u in range(look_g):  # prologue gathers (bf16 cast in DMA)
                gpsimd.indirect_dma_start(
                    out=xraw[:, E * (u % XR) : E * (u % XR + 1)],
                    out_offset=None,
                    in_=d_emb[:, :],
                    in_offset=bass.IndirectOffsetOnAxis(
                        ap=tok[:, u : u + 1], axis=0
                    ),
                ).then_inc(s_gather, 16)

            rdests = [None] + [(0, k) for k in range(1, N_CORES)]
            for t in range(t_steps):
                p = t % 2
                if t + XR < t_steps:
                    # xraw slot (t+XR)%XR free once X(t) transposed
                    gpsimd.wait_ge(s_xtp, t + 1)
                    if t >= 1:
                        # keep the gather off the SDMA engines while our own
                        # previous broadcast drains
                        gpsimd.wait_ge(s_send, 16 * t)
                    gpsimd.indirect_dma_start(
                        out=xraw[:, E * ((t + XR) % XR) : E * ((t + XR) % XR + 1)],
                        out_offset=None,
                        in_=d_emb[:, :],
                        in_offset=bass.IndirectOffsetOnAxis(
                            ap=tok[:, t + XR : t + XR + 1], axis=0
                        ),
                    ).then_inc(s_gather, 16)
                own = hbuf[:, bass.ds(H * p + pid * 128, 128)]
                gpsimd.remote_dma_broadcast(
                    out_ap=own,
                    in_ap=own,
                    remote_sem=arr.sem(),
                    local_sem=s_send,
                    rdests=rdests,
                ).then_inc(s_prep, 1)
                gpsimd.wait_ge(s_prep, t + 1)
                gpsimd.wait_ge(s_htc, t + 1)
                gpsimd.trigger_dma(count=1)

        # ---------------- PE: transposes + matmuls --------------------------
        @block.tensor
        def _(pe):
            pe.wait_ge(s_sync, 160)
            pe.wait_ge(s_gset, 1)
            # weight transposes: Whh 4 groups of 8, Wih 2 groups of 8 (xt_ps
            # as 8-tile scratch)
            for g in range(4):
                if g >= 1:
                    pe.wait_ge(s_wtc, g)
                for j in range(8):
                    ins = nc.tensor.transpose(
                        xt_ps[:, 128 * j : 128 * (j + 1)],
                        stagW[:, H * g + 128 * j : H * g + 128 * (j + 1)],
                        ident[:, :],
                    )
                    if j == 7:
                        ins.then_inc(s_wtp, 1)
            for g2 in range(2):
                pe.wait_ge(s_wtc, 4 + g2)
                for r in range(8):
                    idx = 8 * g2 + r
                    m, e = idx // 4, idx % 4
                    ins = nc.tensor.transpose(
                        xt_ps[:, 128 * r : 128 * (r + 1)],
                        stagI[:, E * m + 128 * e : E * m + 128 * (e + 1)],
                        ident[:, :],
                    )
                    if r == 7:
                        ins.then_inc(s_wtp, 1)

            # X transpose prologue: X(0..look_x-1), bf16
            for u in range(look_x):
                pe.wait_ge(s_gather, 16 * (u + 1))
                if u < 2:
                    pe.wait_ge(s_wtc, 6)  # xt_ps scratch free of weight use
                else:
                    pe.wait_ge(s_xtc, u - 1)  # parity u%2 free (X(u-2) copied)
                for e in range(4):
                    ins = nc.tensor.transpose(
                        xt_ps[:, E * (u % 2) + 128 * e : E * (u % 2) + 128 * (e + 1)],
                        xraw[:, E * (u % XR) + 128 * e : E * (u % XR) + 128 * (e + 1)],
                        ident[:, :],
                    )
                    if e == 3:
                        ins.then_inc(s_xtp, 1)

            pe.wait_ge(s_bias, 1)

            def proj(step, full_stop):
                """bias + X(step) @ WihT into ring bank step%4."""
                r = step % 4
                sl = step % XS
                nc.tensor.matmul(
                    gates_ps[:, GS * r : GS * (r + 1)], ones1[:, :], bias16[:, :],
                    start=True, stop=False, skip_group_check=True,
                )
                for e in range(4):
                    mm = nc.tensor.matmul(
                        gates_ps[:, GS * r : GS * (r + 1)],
                        xt_sb[:, E * sl + 128 * e : E * sl + 128 * (e + 1)],
                        wihT[:, GS * e : GS * (e + 1)],
                        start=False,
                        stop=full_stop and (e == 3),
                        skip_group_check=True,
                    )
                    if full_stop and e == 3:
                        mm.then_inc(s_gA, 1)
                if full_stop:
                    pe.sem_inc(s_gB, 1)

            # prologue projections: step 0 (complete gates: no recurrent term),
            # steps 1-2 (left open for their recurrent matmuls)
            pe.wait_ge(s_xtc, 1)
            proj(0, full_stop=True)
            if t_steps > 1:
                pe.wait_ge(s_xtc, 2)
                proj(1, full_stop=False)

            for t in range(t_steps):
                r = t % 4
                # recurrent matmuls for step t
                if t >= 1:
                    pm = (t - 1) % 2
                    pe.wait_ge(s_htc, t)
                    # staged early wake-ups: burn a warm-keeping matmul after
                    # each partial-arrival milestone so the PE idle gap stays
                    # under the HAM re-throttle window
                    for frac in (10, 6, 3):
                        pe.wait_ge(arr.sem(), max(ARR_PER_STEP * t - frac, 0))
                        nc.tensor.matmul(
                            z_ps[:, 0:GS], msb[:, 0:128], msb[:, 0:GS],
                            start=True, stop=True, skip_group_check=True,
                        )
                    pe.wait_ge(arr.sem(), ARR_PER_STEP * t)
                    for j in range(8):
                        mm = nc.tensor.matmul(
                            gates_ps[:, GS * r : GS * (r + 1)],
                            hbuf[:, H * pm + 128 * j : H * pm + 128 * (j + 1)],
                            whhT[:, GS * j : GS * (j + 1)],
                            start=False,
                            stop=(j == 7),
                            skip_group_check=True,
                        )
                        if j == 7:
                            mm.then_inc(s_gA, 1)
                    pe.sem_inc(s_gB, 1)
                # transpose X(t+3): its gather was issued an iteration ago
                if t + 3 < t_steps:
                    pe.wait_ge(s_gather, 16 * (t + 4))
                    pe.wait_ge(s_xtc, t + 2)  # parity free (X(t+1) copied)
                    for e in range(4):
                        u = t + 3
                        ins = nc.tensor.transpose(
                            xt_ps[:, E * (u % 2) + 128 * e : E * (u % 2) + 128 * (e + 1)],
                            xraw[:, E * (u % XR) + 128 * e : E * (u % XR) + 128 * (e + 1)],
                            ident[:, :],
                        )
                        if e == 3:
                            ins.then_inc(s_xtp, 1)
                # transpose the new h slice as soon as it is ready
                pe.wait_ge(s_h, t + 1)
                p = t % 2
                nc.tensor.transpose(
                    misc_ps[:, 128 * p : 128 * (p + 1)],
                    h_sb[:, 128 * p : 128 * (p + 1)],
                    ident[:, :],
                ).then_inc(s_htp, 1)
                # 1-wide-lhsT separator MM: keeps a non-FWL matmul between the
                # fp32 transpose above and the next 128-wide bf16 weight load
                # (FWL-after-fp32HI is a known HW-hang hazard)
                nc.tensor.matmul(
                    z_ps[:, 0:B], ones1[:, :], bias16[:, 0:B],
                    start=True, stop=True, skip_group_check=True,
                )
                # input projection for step t+2 (filler for the broadcast window)
                if t + 2 < t_steps:
                    pe.wait_ge(s_xtc, t + 3)
                    proj(t + 2, full_stop=False)

            # ---------------- bilinear epilogue ----------------
            pe.wait_ge(arr.sem(), ARR_PER_STEP * t_steps)
            pe.wait_ge(s_htc, t_steps)
            pe.wait_ge(s_gdma, 128)
            for jm in range(8):
                for im in range(8):
                    mm = nc.tensor.matmul(
                        z_ps[:, B * jm : B * (jm + 1)],
                        msb[:, H * im + 128 * jm : H * im + 128 * (jm + 1)],
                        hbuf[:, H * p_last + 128 * im : H * p_last + 128 * im + B],
                        start=(im == 0),
                        stop=(im == 7),
                    )
                    if jm == 7 and im == 7:
                        mm.then_inc(s_z, 1)
            pe.wait_ge(s_zmul, 1)
            for jm in range(8):
                mm = nc.tensor.matmul(
                    misc_ps[0:1, 256:320],
                    ones128[:, :],
                    zw_sb[:, B * jm : B * (jm + 1)],
                    start=(jm == 0),
                    stop=(jm == 7),
                )
                if jm == 7:
                    mm.then_inc(s_zred, 1)

        # ---------------- ACT (scalar): activations ----------------
        @block.scalar
        def _(act):
            import concourse.mybir as mybir

            AF = mybir.ActivationFunctionType
            for t in range(t_steps):
                r = t % 4
                p = t % 2
                act.wait_ge(s_gA, t + 1)
                nc.scalar.activation(
                    sig_sb[:, 384 * p : 384 * p + 256],  # i|f
                    gates_ps[:, GS * r : GS * r + 256],
                    AF.Sigmoid,
                ).then_inc(s_if, 1)
                nc.scalar.activation(
                    tg_sb[:, 128 * p : 128 * (p + 1)],  # g
                    gates_ps[:, GS * r + 256 : GS * r + 384],
                    AF.Tanh,
                ).then_inc(s_tg, 1)
                act.wait_ge(s_gB, t + 1)
                ins = nc.scalar.activation(
                    sig_sb[:, 384 * p + 256 : 384 * p + 384],  # o
                    gates_ps[:, GS * r + 384 : GS * (r + 1)],
                    AF.Sigmoid,
                )
                ins.then_inc(s_so, 1)
                act.wait_ge(s_c, t + 1)
                nc.scalar.activation(
                    tc_sb[:, 128 * p : 128 * (p + 1)],
                    c_sb[:, :],
                    AF.Tanh,
                ).then_inc(s_tc, 1)

            # epilogue sigmoid
            act.wait_ge(s_zred, 1)
            nc.scalar.activation(
                out_sb[:, :], misc_ps[0:1, 256:320], AF.Sigmoid
            ).then_inc(s_out, 1)

        # ---------------- DVE (vector): copies + elementwise ----------------
        @block.vector
        def _(dve):
            dve_pid = dve.partition_id()

            # weight transpose copies (cast f32 -> bf16)
            for g in range(4):
                dve.wait_ge(s_wtp, g + 1)
                for j in range(8):
                    ins = nc.vector.tensor_copy(
                        whhT[:, GS * j + 128 * g : GS * j + 128 * (g + 1)],
                        xt_ps[:, 128 * j : 128 * (j + 1)],
                    )
                    if j == 7:
                        ins.then_inc(s_wtc, 1)
            for g2 in range(2):
                dve.wait_ge(s_wtp, 5 + g2)
                for r in range(8):
                    idx = 8 * g2 + r
                    m, e = idx // 4, idx % 4
                    ins = nc.vector.tensor_copy(
                        wihT[:, GS * e + 128 * m : GS * e + 128 * (m + 1)],
                        xt_ps[:, 128 * r : 128 * (r + 1)],
                    )
                    if r == 7:
                        ins.then_inc(s_wtc, 1)
            # bias add
            dve.wait_ge(s_sync, 288)
            nc.vector.tensor_add(bias[:, :], bias[:, :], btmp[:, :])
            nc.vector.tensor_copy(bias16[:, :], bias[:, :]).then_inc(s_bias, 1)
            # prologue X casts (one 512-wide cast per step)
            for u in range(look_x):
                dve.wait_ge(s_xtp, u + 1)
                nc.vector.tensor_copy(
                    xt_sb[:, E * (u % XS) : E * (u % XS + 1)],
                    xt_ps[:, E * (u % 2) : E * (u % 2 + 1)],
                ).then_inc(s_xtc, 1)
            dve.wait_ge(s_gset, 1)

            for t in range(t_steps):
                p = t % 2
                dve.wait_ge(s_if, t + 1)
                nc.vector.tensor_mul(
                    t1_sb[:, :],
                    sig_sb[:, 384 * p + 128 : 384 * p + 256],  # f
                    c_sb[:, :],
                )
                dve.wait_ge(s_tg, t + 1)
                nc.vector.tensor_mul(
                    t2_sb[:, :],
                    sig_sb[:, 384 * p : 384 * p + 128],  # i
                    tg_sb[:, 128 * p : 128 * (p + 1)],
                )
                nc.vector.tensor_add(c_sb[:, :], t1_sb[:, :], t2_sb[:, :]).then_inc(
                    s_c, 1
                )
                dve.wait_ge(s_tc, t + 1)
                dve.wait_ge(s_so, t + 1)
                nc.vector.tensor_mul(
                    h_sb[:, 128 * p : 128 * (p + 1)],
                    sig_sb[:, 384 * p + 256 : 384 * p + 384],  # o
                    tc_sb[:, 128 * p : 128 * (p + 1)],
                ).then_inc(s_h, 1)
                # cast h^T into hbuf (bf16) once PE transposed it; make sure
                # the send of step t-2 (same parity) has drained first
                dve.wait_ge(s_htp, t + 1)
                if t >= 2:
                    dve.wait_ge(s_send, 16 * (t - 1))
                nc.vector.tensor_copy(
                    hbuf[:, bass.ds(H * p + dve_pid * 128, 128)],
                    misc_ps[:, 128 * p : 128 * (p + 1)],
                ).then_inc(s_htc, 1)
                # cast X(t+3) after the h hand-off (off the critical chain)
                if t + 3 < t_steps:
                    u = t + 3
                    dve.wait_ge(s_xtp, u + 1)
                    nc.vector.tensor_copy(
                        xt_sb[:, E * (u % XS) : E * (u % XS + 1)],
                        xt_ps[:, E * (u % 2) : E * (u % 2 + 1)],
                    ).then_inc(s_xtc, 1)

            # epilogue: rh cast + elementwise mul
            dve.wait_ge(s_z, 1)
            for jm in range(8):
                nc.vector.tensor_copy(
                    rh_sb[:, B * jm : B * (jm + 1)],
                    hbuf[:, H * p_last + 128 * jm + B : H * p_last + 128 * (jm + 1)],
                )
            for jm in range(8):
                ins = nc.vector.tensor_mul(
                    zw_sb[:, B * jm : B * (jm + 1)],
                    z_ps[:, B * jm : B * (jm + 1)],
                    rh_sb[:, B * jm : B * (jm + 1)],
                )
                if jm == 7:
                    ins.then_inc(s_zmul, 1)

    es.close()
    nc.compile()
    return nc


def _get_nc(t_steps=T):
    if t_steps not in _NC_CACHE:
        _NC_CACHE[t_steps] = _build(t_steps)
    return _NC_CACHE[t_steps]


def kernel(**inputs):
    global LAST_EXEC_NS
    from concourse.bass_utils import run_bass_kernel_spmd

    t_steps = int(os.environ.get("BASS_KERNEL_TSTEPS", str(T)))
    nc = _get_nc(t_steps)
    in_map = {
        "contexts": np.ascontiguousarray(np.asarray(inputs["contexts"], np.int32)),
        "responses": np.ascontiguousarray(np.asarray(inputs["responses"], np.int32)),
        "emb": np.ascontiguousarray(np.asarray(inputs["emb"], np.float32)),
        "Wih": np.ascontiguousarray(np.asarray(inputs["Wih"], np.float32)),
        "Whh": np.ascontiguousarray(np.asarray(inputs["Whh"], np.float32)),
        "bih": np.ascontiguousarray(
            np.asarray(inputs["bih"], np.float32).reshape(1, 4 * H)
        ),
        "bhh": np.ascontiguousarray(
            np.asarray(inputs["bhh"], np.float32).reshape(1, 4 * H)
        ),
        "M": np.ascontiguousarray(np.asarray(inputs["M"], np.float32)),
    }
    res = run_bass_kernel_spmd(
        nc,
        [dict(in_map) for _ in range(N_CORES)],
        core_ids=list(range(N_CORES)),
        trace=bool(int(os.environ.get("BASS_KERNEL_TRACE", "0"))),
        trace_cores=(
            list(range(N_CORES))
            if int(os.environ.get("BASS_KERNEL_TRACE_ALL", "0"))
            else None
        ),
    )
    LAST_EXEC_NS = res.exec_time_ns
    return res.results[0]["out"].reshape(B).astype(np.float32)


# revision 15
# speedup vs baseline: 1.0974x; 1.0705x over previous
"""DualEncoder (two shared-weight LSTM encoders + bilinear score) on 8 trn2
NeuronCores.

Sharding: 8-way tensor parallelism over the 4H gate dimension. Every core
holds the full batch (64 ctx + 64 resp sequences = 128 rows) and a 512-wide
gate slice in order [i|f|g|o] (128 each). Per step: gates = bias + x_t @
Wih_k^T + h_{t-1} @ Whh_k^T accumulated in a 4-deep PSUM ring; the input
projection for step t+2 runs ahead of time as PE filler, and staged
warm-keeping matmuls bridge the arrival wait so the PE idle gap stays under
the HAM clock-gate re-throttle window. Activations are widened (one sigmoid
over [i|f], one tanh for g, one sigmoid for o); the new h slice is
PE-transposed, DVE-cast to bf16, and remote-broadcast to every core.
sigmoid(diag(C @ M @ R^T)) is computed replicated at the end.
"""

import os

import numpy as np

N_CORES = 8
B = 64
T = 160
E = 512
H = 1024
V = 32000
GS = 512          # gate-slice width per core
S = 2 * B         # 128 sequences (ctx rows 0:64, resp rows 64:128)
ARR_PER_STEP = 14  # 7 remote senders x 2 sem incs each
XR = 4            # gather prefetch depth (xraw slots)
XS = 3            # transposed-X buffer depth (xt_sb slots)
NA = 384          # i|f|g column group width
NB = 128          # o column group width

LAST_EXEC_NS = None
_NC_CACHE = {}

# gate-chunk order inside the core's 512-wide slice: [i | f | g | o]
GATE_OFF = [0, H, 2 * H, 3 * H]


def _build(t_steps=T):
    from contextlib import ExitStack

    import concourse.bacc as bacc
    import concourse.bass as bass
    import concourse.mybir as mybir
    from concourse import masks

    f32 = mybir.dt.float32
    bf16 = mybir.dt.bfloat16
    i32 = mybir.dt.int32

    nc = bacc.Bacc("TRN2", debug=False, num_devices=N_CORES)

    d_ctx = nc.dram_tensor("contexts", [B, T], i32, kind="ExternalInput")
    d_rsp = nc.dram_tensor("responses", [B, T], i32, kind="ExternalInput")
    d_emb = nc.dram_tensor("emb", [V, E], f32, kind="ExternalInput")
    d_wih = nc.dram_tensor("Wih", [4 * H, E], f32, kind="ExternalInput")
    d_whh = nc.dram_tensor("Whh", [4 * H, H], f32, kind="ExternalInput")
    d_bih = nc.dram_tensor("bih", [1, 4 * H], f32, kind="ExternalInput")
    d_bhh = nc.dram_tensor("bhh", [1, 4 * H], f32, kind="ExternalInput")
    d_m = nc.dram_tensor("M", [H, H], f32, kind="ExternalInput")
    d_out = nc.dram_tensor("out", [1, B], f32, kind="ExternalOutput")

    arr = nc.monotonic_semaphore(0)

    es = ExitStack()
    sb = lambda name, shape, dt: es.enter_context(nc.sbuf_tensor(name, shape, dt))
    psa = lambda name, shape: es.enter_context(nc.psum_tensor(name, shape, f32))
    sem = lambda name: es.enter_context(nc.semaphore(name))

    tok = sb("tok", [S, T], i32)
    whhT = sb("whhT", [128, 8 * GS], bf16)
    wihT = sb("wihT", [128, 4 * GS], bf16)
    msb = sb("msb", [128, 8 * H], bf16)
    stagW = sb("stagW", [128, 4 * H], f32)
    stagI = sb("stagI", [128, 4 * E], f32)
    ident = sb("ident", [128, 128], f32)
    ident16 = sb("ident16", [128, 128], bf16)
    ones1 = sb("ones1", [1, 128], bf16)
    ones128 = sb("ones128", [128, 1], f32)
    bias = sb("bias", [1, GS], f32)
    btmp = sb("btmp", [1, GS], f32)
    bias16 = sb("bias16", [1, GS], bf16)
    xraw = sb("xraw", [128, XR * E], f32)
    xt_sb = sb("xt_sb", [128, XS * E], bf16)
    hbuf = sb("hbuf", [128, 2 * H], bf16)
    h_sb = sb("h_sb", [128, 2 * 128], f32)
    c_sb = sb("c_sb", [128, 128], f32)
    sig_sb = sb("sig_sb", [128, 2 * 384], f32)  # per parity: [i|f|o] 128 each
    tg_sb = sb("tg_sb", [128, 2 * 128], f32)
    tc_sb = sb("tc_sb", [128, 2 * 128], f32)
    t1_sb = sb("t1_sb", [128, 128], f32)
    t2_sb = sb("t2_sb", [128, 128], f32)
    rh_sb = sb("rh_sb", [128, 8 * B], f32)
    zw_sb = sb("zw_sb", [128, 8 * B], f32)
    out_sb = sb("out_sb", [1, B], f32)

    gates_ps = psa("gates_ps", [128, 4 * GS])   # 4-bank ring, bank r = t%4
    xt_ps = psa("xt_ps", [128, 2 * E])          # f32 transpose scratch (2 banks)
    misc_ps = psa("misc_ps", [128, 512])        # ht parities @0/128, s @[0:1,256:320]
    z_ps = psa("z_ps", [128, 8 * B])

    s_sync = sem("s_sync")
    s_gdma = sem("s_gdma")
    s_gset = sem("s_gset")
    s_gather = sem("s_gather")
    s_wtp = sem("s_wtp")
    s_wtc = sem("s_wtc")
    s_bias = sem("s_bias")
    s_xtp = sem("s_xtp")      # X(u) transposed -> u+1
    s_xtc = sem("s_xtc")      # X(u) cast to xt_sb -> u+1
    s_gA = sem("s_gA")        # gates i|f|g of step t done -> t+1
    s_gB = sem("s_gB")        # gates o of step t done -> t+1
    s_if = sem("s_if")
    s_tg = sem("s_tg")
    s_so = sem("s_so")
    s_c = sem("s_c")
    s_tc = sem("s_tc")
    s_h = sem("s_h")
    s_htp = sem("s_htp")
    s_htc = sem("s_htc")
    s_send = sem("s_send")
    s_prep = sem("s_prep")
    s_z = sem("s_z")
    s_zmul = sem("s_zmul")
    s_zred = sem("s_zred")
    s_out = sem("s_out")
    s_fin = sem("s_fin")

    p_last = (t_steps - 1) % 2
    look_g = min(XR, t_steps)
    look_x = min(3, t_steps)

    with nc.Block() as block:

        # ---------------- SYNC (HWDGE): setup loads + final store ----------
        @block.sync
        def _(sync):
            pid = sync.partition_id()
            sync.dma_start(tok[0:B, :], d_ctx[:, :]).then_inc(s_sync, 16)
            sync.dma_start(tok[B:S, :], d_rsp[:, :]).then_inc(s_sync, 16)
            for m in range(4):  # -> 96
                sync.dma_start(
                    stagW[:, H * m : H * (m + 1)],
                    d_whh[bass.ds(pid * 128 + GATE_OFF[m], 128), :],
                ).then_inc(s_sync, 16)
            for m in range(4):  # -> 160
                sync.dma_start(
                    stagI[:, E * m : E * (m + 1)],
                    d_wih[bass.ds(pid * 128 + GATE_OFF[m], 128), :],
                ).then_inc(s_sync, 16)
            for m in range(4):  # -> 224
                sync.dma_start(
                    bias[:, 128 * m : 128 * (m + 1)],
                    d_bih[:, bass.ds(pid * 128 + GATE_OFF[m], 128)],
                ).then_inc(s_sync, 16)
            for m in range(4):  # -> 288
                sync.dma_start(
                    btmp[:, 128 * m : 128 * (m + 1)],
                    d_bhh[:, bass.ds(pid * 128 + GATE_OFF[m], 128)],
                ).then_inc(s_sync, 16)
            sync.wait_ge(s_out, 1)
            sync.dma_start(d_out[:, :], out_sb[:, :]).then_inc(s_fin, 16)
            sync.wait_ge(s_fin, 16)

        # ---------------- GPSIMD: setup, gathers, broadcasts ---------------
        @block.gpsimd
        def _(gpsimd):
            pid = gpsimd.partition_id()
            masks.make_identity(nc, ident[:, :])
            gpsimd.memset(ones1[:, :], 1.0)
            gpsimd.memset(ones128[:, :], 1.0)
            gpsimd.memset(c_sb[:, :], 0.0)
            gpsimd.sem_inc(s_gset, 1)
            for i in range(8):  # M load, f32 -> bf16 cast via SWDGE (-> 128)
                gpsimd.dma_start(
                    msb[:, H * i : H * (i + 1)], d_m[128 * i : 128 * (i + 1), :]
                ).then_inc(s_gdma, 16)

            gpsimd.wait_ge(s_sync, 32)
            for u in range(look_g):  # prologue gathers (bf16 cast in DMA)
                gpsimd.indirect_dma_start(
                    out=xraw[:, E * (u % XR) : E * (u % XR + 1)],
                    out_offset=None,
                    in_=d_emb[:, :],
                    in_offset=bass.IndirectOffsetOnAxis(
                        ap=tok[:, u : u + 1], axis=0
                    ),
                ).then_inc(s_gather, 16)

            rdests = [None] + [(0, k) for k in range(1, N_CORES)]
            for t in range(t_steps):
                p = t % 2
                if t + XR < t_steps:
                    # xraw slot (t+XR)%XR free once X(t) transposed
                    gpsimd.wait_ge(s_xtp, t + 1)
                    if t >= 1:
                        # keep the gather off the SDMA engines while our own
                        # previous broadcast drains
                        gpsimd.wait_ge(s_send, 16 * t)
                    gpsimd.indirect_dma_start(
                        out=xraw[:, E * ((t + XR) % XR) : E * ((t + XR) % XR + 1)],
                        out_offset=None,
                        in_=d_emb[:, :],
                        in_offset=bass.IndirectOffsetOnAxis(
                            ap=tok[:, t + XR : t + XR + 1], axis=0
                        ),
                    ).then_inc(s_gather, 16)
                own = hbuf[:, bass.ds(H * p + pid * 128, 128)]
                gpsimd.remote_dma_broadcast(
                    out_ap=own,
                    in_ap=own,
                    remote_sem=arr.sem(),
                    local_sem=s_send,
                    rdests=rdests,
                ).then_inc(s_prep, 1)
                gpsimd.wait_ge(s_prep, t + 1)
                gpsimd.wait_ge(s_htc, t + 1)
                gpsimd.trigger_dma(count=1)

        # ---------------- PE: transposes + matmuls --------------------------
        @block.tensor
        def _(pe):
            pe.wait_ge(s_sync, 160)
            pe.wait_ge(s_gset, 1)
            # weight transposes: Whh 4 groups of 8, Wih 2 groups of 8 (xt_ps
            # as 8-tile scratch)
            for g in range(4):
                if g >= 1:
                    pe.wait_ge(s_wtc, g)
                for j in range(8):
                    ins = nc.tensor.transpose(
                        xt_ps[:, 128 * j : 128 * (j + 1)],
                        stagW[:, H * g + 128 * j : H * g + 128 * (j + 1)],
                        ident[:, :],
                    )
                    if j == 7:
                        ins.then_inc(s_wtp, 1)
            for g2 in range(2):
                pe.wait_ge(s_wtc, 4 + g2)
                for r in range(8):
                    idx = 8 * g2 + r
                    m, e = idx // 4, idx % 4
                    ins = nc.tensor.transpose(
                        xt_ps[:, 128 * r : 128 * (r + 1)],
                        stagI[:, E * m + 128 * e : E * m + 128 * (e + 1)],
                        ident[:, :],
                    )
                    if r == 7:
                        ins.then_inc(s_wtp, 1)

            # X transpose prologue: X(0..look_x-1), bf16
            for u in range(look_x):
                pe.wait_ge(s_gather, 16 * (u + 1))
                if u < 2:
                    pe.wait_ge(s_wtc, 6)  # xt_ps scratch free of weight use
                else:
                    pe.wait_ge(s_xtc, u - 1)  # parity u%2 free (X(u-2) copied)
                for e in range(4):
                    ins = nc.tensor.transpose(
                        xt_ps[:, E * (u % 2) + 128 * e : E * (u % 2) + 128 * (e + 1)],
                        xraw[:, E * (u % XR) + 128 * e : E * (u % XR) + 128 * (e + 1)],
                        ident[:, :],
                    )
                    if e == 3:
                        ins.then_inc(s_xtp, 1)

            pe.wait_ge(s_bias, 1)

            def proj(step, full_stop):
                """bias + X(step) @ WihT into ring bank step%4."""
                r = step % 4
                sl = step % XS
                nc.tensor.matmul(
                    gates_ps[:, GS * r : GS * (r + 1)], ones1[:, :], bias16[:, :],
                    start=True, stop=False, skip_group_check=True,
                )
                for e in range(4):
                    mm = nc.tensor.matmul(
                        gates_ps[:, GS * r : GS * (r + 1)],
                        xt_sb[:, E * sl + 128 * e : E * sl + 128 * (e + 1)],
                        wihT[:, GS * e : GS * (e + 1)],
                        start=False,
                        stop=full_stop and (e == 3),
                        skip_group_check=True,
                    )
                    if full_stop and e == 3:
                        mm.then_inc(s_gA, 1)
                if full_stop:
                    pe.sem_inc(s_gB, 1)

            # prologue projections: step 0 (complete gates: no recurrent term),
            # steps 1-2 (left open for their recurrent matmuls)
            pe.wait_ge(s_xtc, 1)
            proj(0, full_stop=True)
            if t_steps > 1:
                pe.wait_ge(s_xtc, 2)
                proj(1, full_stop=False)

            for t in range(t_steps):
                r = t % 4
                # recurrent matmuls for step t
                if t >= 1:
                    pm = (t - 1) % 2
                    pe.wait_ge(s_htc, t)
                    # staged early wake-ups: burn a warm-keeping matmul after
                    # each partial-arrival milestone so the PE idle gap stays
                    # under the HAM re-throttle window
                    for frac in (13, 12, 11, 10, 8, 6, 3):
                        pe.wait_ge(arr.sem(), max(ARR_PER_STEP * t - frac, 0))
                        nc.tensor.matmul(
                            z_ps[:, 0:GS], msb[:, 0:128], msb[:, 0:GS],
                            start=True, stop=True, skip_group_check=True,
                        )
                    pe.wait_ge(arr.sem(), ARR_PER_STEP * t)
                    for j in range(8):
                        mm = nc.tensor.matmul(
                            gates_ps[:, GS * r : GS * (r + 1)],
                            hbuf[:, H * pm + 128 * j : H * pm + 128 * (j + 1)],
                            whhT[:, GS * j : GS * (j + 1)],
                            start=False,
                            stop=(j == 7),
                            skip_group_check=True,
                        )
                        if j == 7:
                            mm.then_inc(s_gA, 1)
                    pe.sem_inc(s_gB, 1)
                # transpose X(t+3): its gather was issued an iteration ago
                if t + 3 < t_steps:
                    pe.wait_ge(s_gather, 16 * (t + 4))
                    pe.wait_ge(s_xtc, t + 2)  # parity free (X(t+1) copied)
                    for e in range(4):
                        u = t + 3
                        ins = nc.tensor.transpose(
                            xt_ps[:, E * (u % 2) + 128 * e : E * (u % 2) + 128 * (e + 1)],
                            xraw[:, E * (u % XR) + 128 * e : E * (u % XR) + 128 * (e + 1)],
                            ident[:, :],
                        )
                        if e == 3:
                            ins.then_inc(s_xtp, 1)
                # transpose the new h slice as soon as it is ready
                pe.wait_ge(s_h, t + 1)
                p = t % 2
                nc.tensor.transpose(
                    misc_ps[:, 128 * p : 128 * (p + 1)],
                    h_sb[:, 128 * p : 128 * (p + 1)],
                    ident[:, :],
                ).then_inc(s_htp, 1)
                # 1-wide-lhsT separator MM: keeps a non-FWL matmul between the
                # fp32 transpose above and the next 128-wide bf16 weight load
                # (FWL-after-fp32HI is a known HW-hang hazard)
                nc.tensor.matmul(
                    z_ps[:, 0:B], ones1[:, :], bias16[:, 0:B],
                    start=True, stop=True, skip_group_check=True,
                )
                # input projection for step t+2 (filler for the broadcast window)
                if t + 2 < t_steps:
                    pe.wait_ge(s_xtc, t + 3)
                    proj(t + 2, full_stop=False)
                # unconditional warm-keepers covering the pre-arrival gap
                for _ in range(2):
                    nc.tensor.matmul(
                        z_ps[:, 0:GS], msb[:, 0:128], msb[:, 0:GS],
                        start=True, stop=True, skip_group_check=True,
                    )

            # ---------------- bilinear epilogue ----------------
            pe.wait_ge(arr.sem(), ARR_PER_STEP * t_steps)
            pe.wait_ge(s_htc, t_steps)
            pe.wait_ge(s_gdma, 128)
            for jm in range(8):
                for im in range(8):
                    mm = nc.tensor.matmul(
                        z_ps[:, B * jm : B * (jm + 1)],
                        msb[:, H * im + 128 * jm : H * im + 128 * (jm + 1)],
                        hbuf[:, H * p_last + 128 * im : H * p_last + 128 * im + B],
                        start=(im == 0),
                        stop=(im == 7),
                    )
                    if jm == 7 and im == 7:
                        mm.then_inc(s_z, 1)
            pe.wait_ge(s_zmul, 1)
            for jm in range(8):
                mm = nc.tensor.matmul(
                    misc_ps[0:1, 256:320],
                    ones128[:, :],
                    zw_sb[:, B * jm : B * (jm + 1)],
                    start=(jm == 0),
                    stop=(jm == 7),
                )
                if jm == 7:
                    mm.then_inc(s_zred, 1)

        # ---------------- ACT (scalar): activations ----------------
        @block.scalar
        def _(act):
            import concourse.mybir as mybir

            AF = mybir.ActivationFunctionType
            for t in range(t_steps):
                r = t % 4
                p = t % 2
                act.wait_ge(s_gA, t + 1)
                nc.scalar.activation(
                    sig_sb[:, 384 * p : 384 * p + 256],  # i|f
                    gates_ps[:, GS * r : GS * r + 256],
                    AF.Sigmoid,
                ).then_inc(s_if, 1)
                nc.scalar.activation(
                    tg_sb[:, 128 * p : 128 * (p + 1)],  # g
                    gates_ps[:, GS * r + 256 : GS * r + 384],
                    AF.Tanh,
                ).then_inc(s_tg, 1)
                act.wait_ge(s_gB, t + 1)
                ins = nc.scalar.activation(
                    sig_sb[:, 384 * p + 256 : 384 * p + 384],  # o
                    gates_ps[:, GS * r + 384 : GS * (r + 1)],
                    AF.Sigmoid,
                )
                ins.then_inc(s_so, 1)
                act.wait_ge(s_c, t + 1)
                nc.scalar.activation(
                    tc_sb[:, 128 * p : 128 * (p + 1)],
                    c_sb[:, :],
                    AF.Tanh,
                ).then_inc(s_tc, 1)

            # epilogue sigmoid
            act.wait_ge(s_zred, 1)
            nc.scalar.activation(
                out_sb[:, :], misc_ps[0:1, 256:320], AF.Sigmoid
            ).then_inc(s_out, 1)

        # ---------------- DVE (vector): copies + elementwise ----------------
        @block.vector
        def _(dve):
            dve_pid = dve.partition_id()

            # weight transpose copies (cast f32 -> bf16)
            for g in range(4):
                dve.wait_ge(s_wtp, g + 1)
                for j in range(8):
                    ins = nc.vector.tensor_copy(
                        whhT[:, GS * j + 128 * g : GS * j + 128 * (g + 1)],
                        xt_ps[:, 128 * j : 128 * (j + 1)],
                    )
                    if j == 7:
                        ins.then_inc(s_wtc, 1)
            for g2 in range(2):
                dve.wait_ge(s_wtp, 5 + g2)
                for r in range(8):
                    idx = 8 * g2 + r
                    m, e = idx // 4, idx % 4
                    ins = nc.vector.tensor_copy(
                        wihT[:, GS * e + 128 * m : GS * e + 128 * (m + 1)],
                        xt_ps[:, 128 * r : 128 * (r + 1)],
                    )
                    if r == 7:
                        ins.then_inc(s_wtc, 1)
            # bias add
            dve.wait_ge(s_sync, 288)
            nc.vector.tensor_add(bias[:, :], bias[:, :], btmp[:, :])
            nc.vector.tensor_copy(bias16[:, :], bias[:, :]).then_inc(s_bias, 1)
            # prologue X casts (one 512-wide cast per step)
            for u in range(look_x):
                dve.wait_ge(s_xtp, u + 1)
                nc.vector.tensor_copy(
                    xt_sb[:, E * (u % XS) : E * (u % XS + 1)],
                    xt_ps[:, E * (u % 2) : E * (u % 2 + 1)],
                ).then_inc(s_xtc, 1)
            dve.wait_ge(s_gset, 1)

            for t in range(t_steps):
                p = t % 2
                dve.wait_ge(s_if, t + 1)
                nc.vector.tensor_mul(
                    t1_sb[:, :],
                    sig_sb[:, 384 * p + 128 : 384 * p + 256],  # f
                    c_sb[:, :],
                )
                dve.wait_ge(s_tg, t + 1)
                nc.vector.tensor_mul(
                    t2_sb[:, :],
                    sig_sb[:, 384 * p : 384 * p + 128],  # i
                    tg_sb[:, 128 * p : 128 * (p + 1)],
                )
                nc.vector.tensor_add(c_sb[:, :], t1_sb[:, :], t2_sb[:, :]).then_inc(
                    s_c, 1
                )
                dve.wait_ge(s_tc, t + 1)
                dve.wait_ge(s_so, t + 1)
                nc.vector.tensor_mul(
                    h_sb[:, 128 * p : 128 * (p + 1)],
                    sig_sb[:, 384 * p + 256 : 384 * p + 384],  # o
                    tc_sb[:, 128 * p : 128 * (p + 1)],
                ).then_inc(s_h, 1)
                # cast h^T into hbuf (bf16) once PE transposed it; make sure
                # the send of step t-2 (same parity) has drained first
                dve.wait_ge(s_htp, t + 1)
                if t >= 2:
                    dve.wait_ge(s_send, 16 * (t - 1))
                nc.vector.tensor_copy(
                    hbuf[:, bass.ds(H * p + dve_pid * 128, 128)],
                    misc_ps[:, 128 * p : 128 * (p + 1)],
                ).then_inc(s_htc, 1)
                # cast X(t+3) after the h hand-off (off the critical chain)
                if t + 3 < t_steps:
                    u = t + 3
                    dve.wait_ge(s_xtp, u + 1)
                    nc.vector.tensor_copy(
                        xt_sb[:, E * (u % XS) : E * (u % XS + 1)],
                        xt_ps[:, E * (u % 2) : E * (u % 2 + 1)],
                    ).then_inc(s_xtc, 1)

            # epilogue: rh cast + elementwise mul
            dve.wait_ge(s_z, 1)
            for jm in range(8):
                nc.vector.tensor_copy(
                    rh_sb[:, B * jm : B * (jm + 1)],
                    hbuf[:, H * p_last + 128 * jm + B : H * p_last + 128 * (jm + 1)],
                )
            for jm in range(8):
                ins = nc.vector.tensor_mul(
                    zw_sb[:, B * jm : B * (jm + 1)],
                    z_ps[:, B * jm : B * (jm + 1)],
                    rh_sb[:, B * jm : B * (jm + 1)],
                )
                if jm == 7:
                    ins.then_inc(s_zmul, 1)

    es.close()
    nc.compile()
    return nc


def _get_nc(t_steps=T):
    if t_steps not in _NC_CACHE:
        _NC_CACHE[t_steps] = _build(t_steps)
    return _NC_CACHE[t_steps]


def kernel(**inputs):
    global LAST_EXEC_NS
    from concourse.bass_utils import run_bass_kernel_spmd

    t_steps = int(os.environ.get("BASS_KERNEL_TSTEPS", str(T)))
    nc = _get_nc(t_steps)
    in_map = {
        "contexts": np.ascontiguousarray(np.asarray(inputs["contexts"], np.int32)),
        "responses": np.ascontiguousarray(np.asarray(inputs["responses"], np.int32)),
        "emb": np.ascontiguousarray(np.asarray(inputs["emb"], np.float32)),
        "Wih": np.ascontiguousarray(np.asarray(inputs["Wih"], np.float32)),
        "Whh": np.ascontiguousarray(np.asarray(inputs["Whh"], np.float32)),
        "bih": np.ascontiguousarray(
            np.asarray(inputs["bih"], np.float32).reshape(1, 4 * H)
        ),
        "bhh": np.ascontiguousarray(
            np.asarray(inputs["bhh"], np.float32).reshape(1, 4 * H)
        ),
        "M": np.ascontiguousarray(np.asarray(inputs["M"], np.float32)),
    }
    res = run_bass_kernel_spmd(
        nc,
        [dict(in_map) for _ in range(N_CORES)],
        core_ids=list(range(N_CORES)),
        trace=bool(int(os.environ.get("BASS_KERNEL_TRACE", "0"))),
        trace_cores=(
            list(range(N_CORES))
            if int(os.environ.get("BASS_KERNEL_TRACE_ALL", "0"))
            else None
        ),
    )
    LAST_EXEC_NS = res.exec_time_ns
    return res.results[0]["out"].reshape(B).astype(np.float32)


# revision 16
# speedup vs baseline: 1.1340x; 1.0333x over previous
"""DualEncoder (two shared-weight LSTM encoders + bilinear score) on 8 trn2
NeuronCores.

Sharding: 8-way tensor parallelism over the 4H gate dimension. Every core
holds the full batch (64 ctx + 64 resp sequences = 128 rows) and a 512-wide
gate slice in order [i|f|g|o] (128 each). Per step: gates = bias + x_t @
Wih_k^T + h_{t-1} @ Whh_k^T accumulated in a 4-deep PSUM ring; the input
projection for step t+2 runs ahead of time as PE filler, and staged
warm-keeping matmuls bridge the arrival wait so the PE idle gap stays under
the HAM clock-gate re-throttle window. Activations are widened (one sigmoid
over [i|f], one tanh for g, one sigmoid for o); the new h slice is
PE-transposed, DVE-cast to bf16, and remote-broadcast to every core.
sigmoid(diag(C @ M @ R^T)) is computed replicated at the end.
"""

import os

import numpy as np

N_CORES = 8
B = 64
T = 160
E = 512
H = 1024
V = 32000
GS = 512          # gate-slice width per core
S = 2 * B         # 128 sequences (ctx rows 0:64, resp rows 64:128)
ARR_PER_STEP = 14  # 7 remote senders x 2 sem incs each
XR = 4            # gather prefetch depth (xraw slots)
XS = 3            # transposed-X buffer depth (xt_sb slots)
NA = 384          # i|f|g column group width
NB = 128          # o column group width

LAST_EXEC_NS = None
_NC_CACHE = {}

# gate-chunk order inside the core's 512-wide slice: [i | f | g | o]
GATE_OFF = [0, H, 2 * H, 3 * H]


def _build(t_steps=T):
    from contextlib import ExitStack

    import concourse.bacc as bacc
    import concourse.bass as bass
    import concourse.mybir as mybir
    from concourse import masks

    f32 = mybir.dt.float32
    bf16 = mybir.dt.bfloat16
    i32 = mybir.dt.int32

    nc = bacc.Bacc("TRN2", debug=False, num_devices=N_CORES)

    d_ctx = nc.dram_tensor("contexts", [B, T], i32, kind="ExternalInput")
    d_rsp = nc.dram_tensor("responses", [B, T], i32, kind="ExternalInput")
    d_emb = nc.dram_tensor("emb", [V, E], f32, kind="ExternalInput")
    d_wih = nc.dram_tensor("Wih", [4 * H, E], f32, kind="ExternalInput")
    d_whh = nc.dram_tensor("Whh", [4 * H, H], f32, kind="ExternalInput")
    d_bih = nc.dram_tensor("bih", [1, 4 * H], f32, kind="ExternalInput")
    d_bhh = nc.dram_tensor("bhh", [1, 4 * H], f32, kind="ExternalInput")
    d_m = nc.dram_tensor("M", [H, H], f32, kind="ExternalInput")
    d_out = nc.dram_tensor("out", [1, B], f32, kind="ExternalOutput")

    arr = nc.monotonic_semaphore(0)

    es = ExitStack()
    sb = lambda name, shape, dt: es.enter_context(nc.sbuf_tensor(name, shape, dt))
    psa = lambda name, shape: es.enter_context(nc.psum_tensor(name, shape, f32))
    sem = lambda name: es.enter_context(nc.semaphore(name))

    tok = sb("tok", [S, T], i32)
    whhT = sb("whhT", [128, 8 * GS], bf16)
    wihT = sb("wihT", [128, 4 * GS], bf16)
    msb = sb("msb", [128, 8 * H], bf16)
    stagW = sb("stagW", [128, 4 * H], f32)
    stagI = sb("stagI", [128, 4 * E], f32)
    ident = sb("ident", [128, 128], f32)
    ident16 = sb("ident16", [128, 128], bf16)
    ones1 = sb("ones1", [1, 128], bf16)
    ones128 = sb("ones128", [128, 1], f32)
    bias = sb("bias", [1, GS], f32)
    btmp = sb("btmp", [1, GS], f32)
    bias16 = sb("bias16", [1, GS], bf16)
    xraw = sb("xraw", [128, XR * E], f32)
    xt_sb = sb("xt_sb", [128, XS * E], bf16)
    hbuf = sb("hbuf", [128, 2 * H], bf16)
    h_sb = sb("h_sb", [128, 2 * 128], f32)
    c_sb = sb("c_sb", [128, 128], f32)
    sig_sb = sb("sig_sb", [128, 2 * 384], f32)  # per parity: [i|f|o] 128 each
    tg_sb = sb("tg_sb", [128, 2 * 128], f32)
    tc_sb = sb("tc_sb", [128, 2 * 128], f32)
    t1_sb = sb("t1_sb", [128, 128], f32)
    t2_sb = sb("t2_sb", [128, 128], f32)
    rh_sb = sb("rh_sb", [128, 8 * B], f32)
    zw_sb = sb("zw_sb", [128, 8 * B], f32)
    out_sb = sb("out_sb", [1, B], f32)

    gates_ps = psa("gates_ps", [128, 4 * GS])   # 4-bank ring, bank r = t%4
    xt_ps = psa("xt_ps", [128, 2 * E])          # f32 transpose scratch (2 banks)
    misc_ps = psa("misc_ps", [128, 512])        # ht parities @0/128, s @[0:1,256:320]
    z_ps = psa("z_ps", [128, 8 * B])

    s_sync = sem("s_sync")
    s_gdma = sem("s_gdma")
    s_gset = sem("s_gset")
    s_gather = sem("s_gather")
    s_wtp = sem("s_wtp")
    s_wtc = sem("s_wtc")
    s_bias = sem("s_bias")
    s_xtp = sem("s_xtp")      # X(u) transposed -> u+1
    s_xtc = sem("s_xtc")      # X(u) cast to xt_sb -> u+1
    s_gA = sem("s_gA")        # gates i|f|g of step t done -> t+1
    s_gB = sem("s_gB")        # gates o of step t done -> t+1
    s_if = sem("s_if")
    s_tg = sem("s_tg")
    s_so = sem("s_so")
    s_c = sem("s_c")
    s_tc = sem("s_tc")
    s_h = sem("s_h")
    s_htp = sem("s_htp")
    s_htc = sem("s_htc")
    s_send = sem("s_send")
    s_prep = sem("s_prep")
    s_z = sem("s_z")
    s_zmul = sem("s_zmul")
    s_zred = sem("s_zred")
    s_out = sem("s_out")
    s_fin = sem("s_fin")

    p_last = (t_steps - 1) % 2
    look_g = min(XR, t_steps)
    look_x = min(3, t_steps)

    with nc.Block() as block:

        # ---------------- SYNC (HWDGE): setup loads + final store ----------
        @block.sync
        def _(sync):
            pid = sync.partition_id()
            sync.dma_start(tok[0:B, :], d_ctx[:, :]).then_inc(s_sync, 16)
            sync.dma_start(tok[B:S, :], d_rsp[:, :]).then_inc(s_sync, 16)
            for m in range(4):  # -> 96
                sync.dma_start(
                    stagW[:, H * m : H * (m + 1)],
                    d_whh[bass.ds(pid * 128 + GATE_OFF[m], 128), :],
                ).then_inc(s_sync, 16)
            for m in range(4):  # -> 160
                sync.dma_start(
                    stagI[:, E * m : E * (m + 1)],
                    d_wih[bass.ds(pid * 128 + GATE_OFF[m], 128), :],
                ).then_inc(s_sync, 16)
            for m in range(4):  # -> 224
                sync.dma_start(
                    bias[:, 128 * m : 128 * (m + 1)],
                    d_bih[:, bass.ds(pid * 128 + GATE_OFF[m], 128)],
                ).then_inc(s_sync, 16)
            for m in range(4):  # -> 288
                sync.dma_start(
                    btmp[:, 128 * m : 128 * (m + 1)],
                    d_bhh[:, bass.ds(pid * 128 + GATE_OFF[m], 128)],
                ).then_inc(s_sync, 16)
            sync.wait_ge(s_out, 1)
            sync.dma_start(d_out[:, :], out_sb[:, :]).then_inc(s_fin, 16)
            sync.wait_ge(s_fin, 16)

        # ---------------- GPSIMD: setup, gathers, broadcasts ---------------
        @block.gpsimd
        def _(gpsimd):
            pid = gpsimd.partition_id()
            masks.make_identity(nc, ident[:, :])
            gpsimd.memset(ones1[:, :], 1.0)
            gpsimd.memset(ones128[:, :], 1.0)
            gpsimd.memset(c_sb[:, :], 0.0)
            gpsimd.sem_inc(s_gset, 1)
            for i in range(8):  # M load, f32 -> bf16 cast via SWDGE (-> 128)
                gpsimd.dma_start(
                    msb[:, H * i : H * (i + 1)], d_m[128 * i : 128 * (i + 1), :]
                ).then_inc(s_gdma, 16)

            gpsimd.wait_ge(s_sync, 32)
            for u in range(look_g):  # prologue gathers (bf16 cast in DMA)
                gpsimd.indirect_dma_start(
                    out=xraw[:, E * (u % XR) : E * (u % XR + 1)],
                    out_offset=None,
                    in_=d_emb[:, :],
                    in_offset=bass.IndirectOffsetOnAxis(
                        ap=tok[:, u : u + 1], axis=0
                    ),
                ).then_inc(s_gather, 16)

            rdests = [None] + [(0, k) for k in range(1, N_CORES)]
            for t in range(t_steps):
                p = t % 2
                if t + XR < t_steps:
                    # xraw slot (t+XR)%XR free once X(t) transposed
                    gpsimd.wait_ge(s_xtp, t + 1)
                    if t >= 1:
                        # keep the gather off the SDMA engines while our own
                        # previous broadcast drains
                        gpsimd.wait_ge(s_send, 16 * t)
                    gpsimd.indirect_dma_start(
                        out=xraw[:, E * ((t + XR) % XR) : E * ((t + XR) % XR + 1)],
                        out_offset=None,
                        in_=d_emb[:, :],
                        in_offset=bass.IndirectOffsetOnAxis(
                            ap=tok[:, t + XR : t + XR + 1], axis=0
                        ),
                    ).then_inc(s_gather, 16)
                own = hbuf[:, bass.ds(H * p + pid * 128, 128)]
                gpsimd.remote_dma_broadcast(
                    out_ap=own,
                    in_ap=own,
                    remote_sem=arr.sem(),
                    local_sem=s_send,
                    rdests=rdests,
                ).then_inc(s_prep, 1)
                gpsimd.wait_ge(s_prep, t + 1)
                gpsimd.wait_ge(s_htc, t + 1)
                gpsimd.trigger_dma(count=1)

        # ---------------- PE: transposes + matmuls --------------------------
        @block.tensor
        def _(pe):
            pe.wait_ge(s_sync, 96)   # stagW loaded (tok 32 + 4x whh rows)
            pe.wait_ge(s_gset, 1)
            # weight transposes: Whh 4 groups of 8, Wih 2 groups of 8 (xt_ps
            # as 8-tile scratch)
            for g in range(4):
                if g >= 1:
                    pe.wait_ge(s_wtc, g)
                for j in range(8):
                    ins = nc.tensor.transpose(
                        xt_ps[:, 128 * j : 128 * (j + 1)],
                        stagW[:, H * g + 128 * j : H * g + 128 * (j + 1)],
                        ident[:, :],
                    )
                    if j == 7:
                        ins.then_inc(s_wtp, 1)
            pe.wait_ge(s_sync, 160)  # stagI loaded
            for g2 in range(2):
                pe.wait_ge(s_wtc, 4 + g2)
                for r in range(8):
                    idx = 8 * g2 + r
                    m, e = idx // 4, idx % 4
                    ins = nc.tensor.transpose(
                        xt_ps[:, 128 * r : 128 * (r + 1)],
                        stagI[:, E * m + 128 * e : E * m + 128 * (e + 1)],
                        ident[:, :],
                    )
                    if r == 7:
                        ins.then_inc(s_wtp, 1)

            # X transpose prologue: X(0..look_x-1), bf16
            for u in range(look_x):
                pe.wait_ge(s_gather, 16 * (u + 1))
                if u < 2:
                    pe.wait_ge(s_wtc, 6)  # xt_ps scratch free of weight use
                else:
                    pe.wait_ge(s_xtc, u - 1)  # parity u%2 free (X(u-2) copied)
                for e in range(4):
                    ins = nc.tensor.transpose(
                        xt_ps[:, E * (u % 2) + 128 * e : E * (u % 2) + 128 * (e + 1)],
                        xraw[:, E * (u % XR) + 128 * e : E * (u % XR) + 128 * (e + 1)],
                        ident[:, :],
                    )
                    if e == 3:
                        ins.then_inc(s_xtp, 1)

            pe.wait_ge(s_bias, 1)

            def proj(step, full_stop):
                """bias + X(step) @ WihT into ring bank step%4."""
                r = step % 4
                sl = step % XS
                nc.tensor.matmul(
                    gates_ps[:, GS * r : GS * (r + 1)], ones1[:, :], bias16[:, :],
                    start=True, stop=False, skip_group_check=True,
                )
                for e in range(4):
                    mm = nc.tensor.matmul(
                        gates_ps[:, GS * r : GS * (r + 1)],
                        xt_sb[:, E * sl + 128 * e : E * sl + 128 * (e + 1)],
                        wihT[:, GS * e : GS * (e + 1)],
                        start=False,
                        stop=full_stop and (e == 3),
                        skip_group_check=True,
                    )
                    if full_stop and e == 3:
                        mm.then_inc(s_gA, 1)
                if full_stop:
                    pe.sem_inc(s_gB, 1)

            # prologue projections: step 0 (complete gates: no recurrent term),
            # steps 1-2 (left open for their recurrent matmuls)
            pe.wait_ge(s_xtc, 1)
            proj(0, full_stop=True)
            if t_steps > 1:
                pe.wait_ge(s_xtc, 2)
                proj(1, full_stop=False)

            for t in range(t_steps):
                r = t % 4
                # recurrent matmuls for step t
                if t >= 1:
                    pm = (t - 1) % 2
                    pe.wait_ge(s_htc, t)
                    # staged early wake-ups: burn a warm-keeping matmul after
                    # each partial-arrival milestone so the PE idle gap stays
                    # under the HAM re-throttle window
                    for frac in (13, 12, 11, 10, 8, 6, 3):
                        pe.wait_ge(arr.sem(), max(ARR_PER_STEP * t - frac, 0))
                        nc.tensor.matmul(
                            z_ps[:, 0:GS], msb[:, 0:128], msb[:, 0:GS],
                            start=True, stop=True, skip_group_check=True,
                        )
                    pe.wait_ge(arr.sem(), ARR_PER_STEP * t)
                    for j in range(8):
                        mm = nc.tensor.matmul(
                            gates_ps[:, GS * r : GS * (r + 1)],
                            hbuf[:, H * pm + 128 * j : H * pm + 128 * (j + 1)],
                            whhT[:, GS * j : GS * (j + 1)],
                            start=False,
                            stop=(j == 7),
                            skip_group_check=True,
                        )
                        if j == 7:
                            mm.then_inc(s_gA, 1)
                    pe.sem_inc(s_gB, 1)
                # transpose X(t+3): its gather was issued an iteration ago
                if t + 3 < t_steps:
                    pe.wait_ge(s_gather, 16 * (t + 4))
                    pe.wait_ge(s_xtc, t + 2)  # parity free (X(t+1) copied)
                    for e in range(4):
                        u = t + 3
                        ins = nc.tensor.transpose(
                            xt_ps[:, E * (u % 2) + 128 * e : E * (u % 2) + 128 * (e + 1)],
                            xraw[:, E * (u % XR) + 128 * e : E * (u % XR) + 128 * (e + 1)],
                            ident[:, :],
                        )
                        if e == 3:
                            ins.then_inc(s_xtp, 1)
                # warm-keepers covering the cell-update window (they finish
                # well before s_h fires, so the h transpose is not delayed)
                if t >= 1:
                    for _ in range(4):
                        nc.tensor.matmul(
                            z_ps[:, 0:GS], msb[:, 0:128], msb[:, 0:GS],
                            start=True, stop=True, skip_group_check=True,
                        )
                # transpose the new h slice as soon as it is ready
                pe.wait_ge(s_h, t + 1)
                p = t % 2
                nc.tensor.transpose(
                    misc_ps[:, 128 * p : 128 * (p + 1)],
                    h_sb[:, 128 * p : 128 * (p + 1)],
                    ident[:, :],
                ).then_inc(s_htp, 1)
                # 1-wide-lhsT separator MM: keeps a non-FWL matmul between the
                # fp32 transpose above and the next 128-wide bf16 weight load
                # (FWL-after-fp32HI is a known HW-hang hazard)
                nc.tensor.matmul(
                    z_ps[:, 0:B], ones1[:, :], bias16[:, 0:B],
                    start=True, stop=True, skip_group_check=True,
                )
                # input projection for step t+2 (filler for the broadcast window)
                if t + 2 < t_steps:
                    pe.wait_ge(s_xtc, t + 3)
                    proj(t + 2, full_stop=False)
                # unconditional warm-keepers covering the pre-arrival gap
                for _ in range(2):
                    nc.tensor.matmul(
                        z_ps[:, 0:GS], msb[:, 0:128], msb[:, 0:GS],
                        start=True, stop=True, skip_group_check=True,
                    )

            # ---------------- bilinear epilogue ----------------
            pe.wait_ge(arr.sem(), ARR_PER_STEP * t_steps)
            pe.wait_ge(s_htc, t_steps)
            pe.wait_ge(s_gdma, 128)
            for jm in range(8):
                for im in range(8):
                    mm = nc.tensor.matmul(
                        z_ps[:, B * jm : B * (jm + 1)],
                        msb[:, H * im + 128 * jm : H * im + 128 * (jm + 1)],
                        hbuf[:, H * p_last + 128 * im : H * p_last + 128 * im + B],
                        start=(im == 0),
                        stop=(im == 7),
                    )
                    if jm == 7 and im == 7:
                        mm.then_inc(s_z, 1)
            pe.wait_ge(s_zmul, 1)
            for jm in range(8):
                mm = nc.tensor.matmul(
                    misc_ps[0:1, 256:320],
                    ones128[:, :],
                    zw_sb[:, B * jm : B * (jm + 1)],
                    start=(jm == 0),
                    stop=(jm == 7),
                )
                if jm == 7:
                    mm.then_inc(s_zred, 1)

        # ---------------- ACT (scalar): activations ----------------
        @block.scalar
        def _(act):
            import concourse.mybir as mybir

            AF = mybir.ActivationFunctionType
            for t in range(t_steps):
                r = t % 4
                p = t % 2
                act.wait_ge(s_gA, t + 1)
                nc.scalar.activation(
                    sig_sb[:, 384 * p : 384 * p + 256],  # i|f
                    gates_ps[:, GS * r : GS * r + 256],
                    AF.Sigmoid,
                ).then_inc(s_if, 1)
                nc.scalar.activation(
                    tg_sb[:, 128 * p : 128 * (p + 1)],  # g
                    gates_ps[:, GS * r + 256 : GS * r + 384],
                    AF.Tanh,
                ).then_inc(s_tg, 1)
                act.wait_ge(s_gB, t + 1)
                ins = nc.scalar.activation(
                    sig_sb[:, 384 * p + 256 : 384 * p + 384],  # o
                    gates_ps[:, GS * r + 384 : GS * (r + 1)],
                    AF.Sigmoid,
                )
                ins.then_inc(s_so, 1)
                act.wait_ge(s_c, t + 1)
                nc.scalar.activation(
                    tc_sb[:, 128 * p : 128 * (p + 1)],
                    c_sb[:, :],
                    AF.Tanh,
                ).then_inc(s_tc, 1)

            # epilogue sigmoid
            act.wait_ge(s_zred, 1)
            nc.scalar.activation(
                out_sb[:, :], misc_ps[0:1, 256:320], AF.Sigmoid
            ).then_inc(s_out, 1)

        # ---------------- DVE (vector): copies + elementwise ----------------
        @block.vector
        def _(dve):
            dve_pid = dve.partition_id()

            # weight transpose copies (cast f32 -> bf16)
            for g in range(4):
                dve.wait_ge(s_wtp, g + 1)
                for j in range(8):
                    ins = nc.vector.tensor_copy(
                        whhT[:, GS * j + 128 * g : GS * j + 128 * (g + 1)],
                        xt_ps[:, 128 * j : 128 * (j + 1)],
                    )
                    if j == 7:
                        ins.then_inc(s_wtc, 1)
            for g2 in range(2):
                dve.wait_ge(s_wtp, 5 + g2)
                for r in range(8):
                    idx = 8 * g2 + r
                    m, e = idx // 4, idx % 4
                    ins = nc.vector.tensor_copy(
                        wihT[:, GS * e + 128 * m : GS * e + 128 * (m + 1)],
                        xt_ps[:, 128 * r : 128 * (r + 1)],
                    )
                    if r == 7:
                        ins.then_inc(s_wtc, 1)
            # bias add
            dve.wait_ge(s_sync, 288)
            nc.vector.tensor_add(bias[:, :], bias[:, :], btmp[:, :])
            nc.vector.tensor_copy(bias16[:, :], bias[:, :]).then_inc(s_bias, 1)
            # prologue X casts (one 512-wide cast per step)
            for u in range(look_x):
                dve.wait_ge(s_xtp, u + 1)
                nc.vector.tensor_copy(
                    xt_sb[:, E * (u % XS) : E * (u % XS + 1)],
                    xt_ps[:, E * (u % 2) : E * (u % 2 + 1)],
                ).then_inc(s_xtc, 1)
            dve.wait_ge(s_gset, 1)

            for t in range(t_steps):
                p = t % 2
                dve.wait_ge(s_if, t + 1)
                nc.vector.tensor_mul(
                    t1_sb[:, :],
                    sig_sb[:, 384 * p + 128 : 384 * p + 256],  # f
                    c_sb[:, :],
                )
                dve.wait_ge(s_tg, t + 1)
                nc.vector.tensor_mul(
                    t2_sb[:, :],
                    sig_sb[:, 384 * p : 384 * p + 128],  # i
                    tg_sb[:, 128 * p : 128 * (p + 1)],
                )
                nc.vector.tensor_add(c_sb[:, :], t1_sb[:, :], t2_sb[:, :]).then_inc(
                    s_c, 1
                )
                dve.wait_ge(s_tc, t + 1)
                dve.wait_ge(s_so, t + 1)
                nc.vector.tensor_mul(
                    h_sb[:, 128 * p : 128 * (p + 1)],
                    sig_sb[:, 384 * p + 256 : 384 * p + 384],  # o
                    tc_sb[:, 128 * p : 128 * (p + 1)],
                ).then_inc(s_h, 1)
                # cast h^T into hbuf (bf16) once PE transposed it; make sure
                # the send of step t-2 (same parity) has drained first
                dve.wait_ge(s_htp, t + 1)
                if t >= 2:
                    dve.wait_ge(s_send, 16 * (t - 1))
                nc.vector.tensor_copy(
                    hbuf[:, bass.ds(H * p + dve_pid * 128, 128)],
                    misc_ps[:, 128 * p : 128 * (p + 1)],
                ).then_inc(s_htc, 1)
                # cast X(t+3) after the h hand-off (off the critical chain)
                if t + 3 < t_steps:
                    u = t + 3
                    dve.wait_ge(s_xtp, u + 1)
                    nc.vector.tensor_copy(
                        xt_sb[:, E * (u % XS) : E * (u % XS + 1)],
                        xt_ps[:, E * (u % 2) : E * (u % 2 + 1)],
                    ).then_inc(s_xtc, 1)

            # epilogue: rh cast + elementwise mul
            dve.wait_ge(s_z, 1)
            for jm in range(8):
                nc.vector.tensor_copy(
                    rh_sb[:, B * jm : B * (jm + 1)],
                    hbuf[:, H * p_last + 128 * jm + B : H * p_last + 128 * (jm + 1)],
                )
            for jm in range(8):
                ins = nc.vector.tensor_mul(
                    zw_sb[:, B * jm : B * (jm + 1)],
                    z_ps[:, B * jm : B * (jm + 1)],
                    rh_sb[:, B * jm : B * (jm + 1)],
                )
                if jm == 7:
                    ins.then_inc(s_zmul, 1)

    es.close()
    nc.compile()
    return nc


def _get_nc(t_steps=T):
    if t_steps not in _NC_CACHE:
        _NC_CACHE[t_steps] = _build(t_steps)
    return _NC_CACHE[t_steps]


def kernel(**inputs):
    global LAST_EXEC_NS
    from concourse.bass_utils import run_bass_kernel_spmd

    t_steps = int(os.environ.get("BASS_KERNEL_TSTEPS", str(T)))
    nc = _get_nc(t_steps)
    in_map = {
        "contexts": np.ascontiguousarray(np.asarray(inputs["contexts"], np.int32)),
        "responses": np.ascontiguousarray(np.asarray(inputs["responses"], np.int32)),
        "emb": np.ascontiguousarray(np.asarray(inputs["emb"], np.float32)),
        "Wih": np.ascontiguousarray(np.asarray(inputs["Wih"], np.float32)),
        "Whh": np.ascontiguousarray(np.asarray(inputs["Whh"], np.float32)),
        "bih": np.ascontiguousarray(
            np.asarray(inputs["bih"], np.float32).reshape(1, 4 * H)
        ),
        "bhh": np.ascontiguousarray(
            np.asarray(inputs["bhh"], np.float32).reshape(1, 4 * H)
        ),
        "M": np.ascontiguousarray(np.asarray(inputs["M"], np.float32)),
    }
    res = run_bass_kernel_spmd(
        nc,
        [dict(in_map) for _ in range(N_CORES)],
        core_ids=list(range(N_CORES)),
        trace=bool(int(os.environ.get("BASS_KERNEL_TRACE", "0"))),
        trace_cores=(
            list(range(N_CORES))
            if int(os.environ.get("BASS_KERNEL_TRACE_ALL", "0"))
            else None
        ),
    )
    LAST_EXEC_NS = res.exec_time_ns
    return res.results[0]["out"].reshape(B).astype(np.float32)
